# revision 42
# baseline (speedup 1.0000x reference)
"""Self-attention scores kernel for Trainium2, 8-core SPMD.

Computes softmax((x@Wq+bq) @ (x@Wq+bq)^T / sqrt(64)) per head
(reference reuses the query projection for k, bug-for-bug).

Sharding: 32 (batch, head) pairs split 4-per-core across 8 cores.
Core c handles batch c//4, heads 4*(c%4) .. 4*(c%4)+3.

Pipeline per 128-row block ([128, 2048] f32 PSUM tile, 4x N=512
matmuls): ScalarE computes exp(z/8 - 9) -> fp16 SBUF in one big
ACTIVATE (row sums fall out of the ACT accumulator); VectorE
reciprocals and normalizes in-place (fp16 4x mode); 1 MiB fp16 stores
stream out per 2-block group.  A garbage-matmul warm-up burst at t=0
un-throttles the PE clock (HAM) before the projections; x^T streams in
as 16 column-major 256 KiB chunks on both HWDGE queues, and the two
head-pairs' projections are emitted chunk-by-chunk in input-arrival
order so all eight run warm inside the prologue (no mid-kernel qt1
stall).  Host converts fp16 -> f32 (the unshard step).
"""

import numpy as np

import concourse.bass as bass
import concourse.mybir as mybir
import concourse.tile as tile
from concourse import bacc
from concourse.bass_utils import run_bass_kernel_spmd

B = 2
S = 2048
D = 1024
H = 16
HS = 64
N_CORES = 8
HEADS_PER_CORE = 4  # 2 pairs of 2 heads (pair = 128 partitions)
KK = D // 128  # 8 k-tiles for the projection contraction
NQ = S // 128  # 16 q row-blocks per head

IN_DT = mybir.dt.float16
F16 = mybir.dt.float16
F32 = mybir.dt.float32

# exp(z - SHIFT) keeps all values in fp16 range: z = q.k/8 <= max|q|^2/8,
# which concentrates near 8 and exceeds SHIFT + 11 (fp16 overflow) with
# probability ~1e-8. Softmax normalization cancels the shift exactly.
SHIFT = 9.0


def _build():
    nc = bacc.Bacc("TRN2", target_bir_lowering=False, debug=False)
    # x^T in column-half-major layout [2, D, 1024] so each [128, 1024]
    # SBUF chunk is a contiguous 256 KiB DRAM read.
    xT = nc.dram_tensor("xT", [2, D, 1024], IN_DT, kind="ExternalInput").ap()
    WqS = nc.dram_tensor("WqS", [D, HEADS_PER_CORE * HS], IN_DT, kind="ExternalInput").ap()
    bqS = nc.dram_tensor("bqS", [128, 3], F32, kind="ExternalInput").ap()
    out = nc.dram_tensor("out", [HEADS_PER_CORE, S, S], F16, kind="ExternalOutput").ap()

    with tile.TileContext(nc) as tc:
        with (
            tc.tile_pool(name="consts", bufs=1) as consts,
            tc.tile_pool(name="qt", bufs=2) as qt_pool,
            tc.tile_pool(name="xt", bufs=2 * KK) as xt_pool,
            tc.tile_pool(name="ps", bufs=2, space="PSUM") as ps_pool,
            tc.tile_pool(name="et", bufs=8) as et_pool,
            tc.tile_pool(name="small", bufs=8) as small,
        ):
            w = consts.tile([128, KK, HEADS_PER_CORE * HS], IN_DT)
            nc.sync.dma_start(out=w[:], in_=WqS.rearrange("(kk p) c -> p kk c", p=128))
            # bias columns 0-1 = per-pair projection bias; column 2 is
            # the baked -SHIFT constant for the exp bias.
            bias = consts.tile([128, 3], F32)
            nc.sync.dma_start(out=bias[:], in_=bqS)
            shift = bias[:, 2:3]

            # PE warm-up: a dense burst of garbage matmuls with no input
            # dependencies keeps the HAM activity monitor busy (~3.4us
            # sustained) so the projection runs at 2.4 GHz instead of the
            # cold 1.2 GHz default.  Runs while the input DMAs stream.
            gz = consts.tile([128, 512], IN_DT)
            nc.vector.memset(gz[:], 0.0)
            # tiny dummy exp: pulls ACT_TABLE_LOAD (~1.3-2.7us) off the
            # first real ACTIVATE's critical path (writes a scratch cell
            # so the PE warm-up matmuls reading gz don't wait on it)
            gsc = consts.tile([128, 1], F32)
            nc.scalar.activation(
                out=gsc[:],
                in_=gz[:, 0:1],
                func=mybir.ActivationFunctionType.Exp,
            )
            psw = ps_pool.tile([128, S], F32, tag="ps", name="psw")
            for _ in range(30):
                nc.tensor.matmul(
                    psw[:, 0:512],
                    lhsT=gz[:, 0:128],
                    rhs=gz[:],
                    start=True,
                    stop=True,
                )

            # x^T streamed as 16 independent [128, 1024] chunks, emitted
            # column-major so projection chunk n can start after just the
            # 8 k-tiles of its column range have landed.
            xts = [[None] * 2 for _ in range(KK)]
            for n2 in range(2):
                for kk in range(KK):
                    xtt = xt_pool.tile([128, 1024], IN_DT, tag="xt", name="xt")
                    eng = nc.sync if kk % 2 == 0 else nc.scalar
                    eng.dma_start(
                        out=xtt[:],
                        in_=xT[n2, kk * 128 : (kk + 1) * 128, :],
                    )
                    xts[kk][n2] = xtt

            # ---- Projection, emitted chunk-by-chunk so pair-1 half-0
            # chunks fill the PE while x's second half still streams in ----
            def project_chunk(g, qtg, n):
                ps = ps_pool.tile([128, S], F32, tag="ps", name="psp")
                for kk in range(KK):
                    nc.tensor.matmul(
                        ps[:, n * 512 : (n + 1) * 512],
                        lhsT=w[:, kk, g * 128 : (g + 1) * 128],
                        rhs=xts[kk][n // 2][:, (n % 2) * 512 : (n % 2 + 1) * 512],
                        start=(kk == 0),
                        stop=(kk == KK - 1),
                    )
                nc.vector.tensor_scalar_add(
                    qtg[:, n * 512 : (n + 1) * 512],
                    ps[:, n * 512 : (n + 1) * 512],
                    bias[:, g : g + 1],
                )

            # ---- Scores + softmax for one head, 2 row-blocks per DMA ----
            def score_head(h, qtg, ipa=0, ipb=NQ // 2):
                pb = (h % 2) * 64
                for ip in range(ipa, ipb):
                    et = et_pool.tile([128, 2, S], F16, tag="et")
                    for r in range(2):
                        i = 2 * ip + r
                        lhsT = qtg[pb : pb + 64, i * 128 : (i + 1) * 128]
                        ps = ps_pool.tile([128, S], F32, tag="ps", name="pss")
                        for j in range(4):
                            nc.tensor.matmul(
                                ps[:, j * 512 : (j + 1) * 512],
                                lhsT=lhsT,
                                rhs=qtg[pb : pb + 64, j * 512 : (j + 1) * 512],
                                start=True,
                                stop=True,
                            )
                        sums = small.tile([128, 1], F32, tag="sm", name="sm")
                        nc.scalar.activation(
                            out=et[:, r, :],
                            in_=ps[:],
                            func=mybir.ActivationFunctionType.Exp,
                            scale=1.0 / np.sqrt(float(HS)),
                            bias=shift,
                            accum_out=sums[:],
                        )
                        recip = small.tile([128, 1], F32, tag="rc", name="rc")
                        nc.vector.reciprocal(recip[:], sums[:])
                        nc.vector.tensor_scalar_mul(et[:, r, :], et[:, r, :], recip[:])
                    if h == HEADS_PER_CORE - 1 and ip >= NQ // 2 - 2:
                        # tail: store per-block so the final drain overlaps
                        for r in range(2):
                            nc.sync.dma_start(
                                out=out[
                                    h,
                                    (2 * ip + r) * 128 : (2 * ip + r + 1) * 128,
                                    :,
                                ],
                                in_=et[:, r, :],
                            )
                    else:
                        nc.sync.dma_start(
                            out=out[h, ip * 256 : (ip + 1) * 256, :].rearrange(
                                "(r p) c -> p r c", p=128
                            ),
                            in_=et[:],
                        )

            qt0 = qt_pool.tile([128, S], F16, tag="qt", name="qt0")
            qt1 = qt_pool.tile([128, S], F16, tag="qt", name="qt1")
            for g, qtg, n in (
                (0, qt0, 0),
                (0, qt0, 1),
                (1, qt1, 0),
                (1, qt1, 1),
                (0, qt0, 2),
                (0, qt0, 3),
            ):
                project_chunk(g, qtg, n)
            # pair-1's last chunks hide under head-0's first two
            # ACTIVATEs while the PE is still HAM-warm
            score_head(0, qt0, 0, 1)
            project_chunk(1, qt1, 2)
            project_chunk(1, qt1, 3)
            score_head(0, qt0, 1)
            score_head(1, qt0)
            score_head(2, qt1)
            score_head(3, qt1)
    nc.compile()
    return nc


_NC_CACHE = None


def kernel(x, Wq, bq):
    global _NC_CACHE
    x = np.asarray(x, dtype=np.float32)
    Wq = np.asarray(Wq, dtype=np.float32)
    bq = np.asarray(bq, dtype=np.float32)
    assert x.shape == (B, S, D) and Wq.shape == (D, D) and bq.shape == (D,)

    if _NC_CACHE is None:
        _NC_CACHE = _build()
    nc = _NC_CACHE

    xTs = [
        np.ascontiguousarray(
            x[b].T.astype(np.float16).reshape(D, 2, 1024).transpose(1, 0, 2)
        )
        for b in range(B)
    ]
    Wq16 = Wq.astype(np.float16)
    in_maps = []
    for c in range(N_CORES):
        b, hg = divmod(c, N_CORES // B)
        h0 = hg * HEADS_PER_CORE
        in_maps.append(
            {
                "xT": xTs[b],
                "WqS": np.ascontiguousarray(Wq16[:, h0 * HS : (h0 + HEADS_PER_CORE) * HS]),
                "bqS": np.ascontiguousarray(
                    np.concatenate(
                        [
                            bq[h0 * HS : (h0 + HEADS_PER_CORE) * HS].reshape(2, 128).T,
                            np.full((128, 1), -SHIFT, dtype=np.float32),
                        ],
                        axis=1,
                    )
                ),
            }
        )

    res = run_bass_kernel_spmd(nc, in_maps, core_ids=list(range(N_CORES)))

    full = np.empty((B, H, S, S), dtype=np.float32)
    for c in range(N_CORES):
        b, hg = divmod(c, N_CORES // B)
        h0 = hg * HEADS_PER_CORE
        full[b, h0 : h0 + HEADS_PER_CORE] = res.results[c]["out"]
    return full
